# revision 27
# baseline (speedup 1.0000x reference)
"""VQ codebook lookup kernel for Trainium2 (8 NeuronCores, data-parallel).

Computes out[b] = values[argmin_k ||x[b] - keys[k]||] for
x [65536, 512], keys/values [1024, 512] fp32.

Strategy (per core, batch shard of 8192 rows):
  - argmin distance == argmax s = 2*x.k - |k|^2 (sqrt and the |x|^2 row
    offset do not change the argmin).
  - Per 128-row tile, s[128, 1024] accumulates in PSUM from:
      1. ScalarE pre-writes the exact f32 -|k|^2 bias into the PSUM tile;
         all matmuls use start=False and accumulate onto it (warmup
         matmuls pre-set the has_written bits of the 4 rotating PSUM
         buffers once, so the PE accumulates instead of overwriting).
      2. 8x fp16 matmuls (K=128, N=512): fp16(x).T @ fp16(2k) hi*hi term.
         The PE multiplies fp16 operands exactly (fp22 internally) and
         accumulates in fp32.
      3. 8x fp8-e4m3 DoubleRow matmuls (K=256 packed, N=512) for the two
         cross terms: fp8(xl*64) . fp8(2k/64) and fp8(x/32) . fp8(kl*32),
         where xl = x - fp16(x), kl = 2k - fp16(2k); scales keep the
         operands inside e4m3 range. DoubleRow packs 2 contraction rows
         per PE cell, halving the matmul count for these passes.
    A matmul instruction costs ~N cycles regardless of dtype, so the tile
    costs 16 matmuls vs 24 for the bf16 hi/lo x3 scheme (1.5x less PE
    time); matmuls are grouped by PE mode (fp16, then DoubleRow) to
    minimize mode-switch stalls. Numpy-sim of this exact arithmetic
    flips 1 of 65536 argmaxes (rel err 5.5e-3, gate 2e-2).
  - DVE per tile: MAX8 + FIND_INDEX8 read straight from PSUM (no
    PSUM->SBUF move), then a gpsimd indirect-DMA gathers fp16 values
    rows and the result is written out from the Scalar queue; the host
    upcasts the fp16 output to f32 (values fp16 rounding adds ~1e-3 rel,
    far under the gate, and halves the gather+store HBM traffic).

Schedule (worth ~8us vs the naive version): all input loads share one
DMA queue in tile0's consumption order so the bulk x prefetch cannot
starve tile0's operands; the PSUM pre-touches run first (no DMA deps,
hidden under the ~4us DMA-ring bring-up); the bias ships as a 4KB row
that two cold K=1 fp32 matmuls broadcast across partitions (doubling
as HAM clock warmup) with tile0 accumulating directly onto that PSUM
tile; 4 rotating PSUM score buffers (all 8 banks) decouple the PE from
the DVE argmax + gather pipeline; leading x blocks are small
(128/128/256 rows) so early tiles aren't gated on 2MB transfers while
the DMA rings ramp; output stores alternate between the two HWDGE
queues so the final tiles' stores drain in parallel.

Measured on TRN2: HW exec ~251.5us (baseline 259.1us), rel err
5.47e-3, PE matmul stream ~99% dense at the 1-column/cycle roofline.
Note the chip sometimes drops to a lower power state under sustained
load (every engine ~18% slower); timings are from unthrottled runs
(steady-state N=512 matmul ~216ns; ~259ns when throttled).
"""

import numpy as np

_B = 65536
_D = 512
_K = 1024
_NCORES = 8
_BL = _B // _NCORES  # 8192 rows per core
_P = 128
_BBLK = 512          # b columns loaded per DMA
_BT = 128            # b rows per matmul tile (PSUM partition dim)
_DC = _D // _P       # 4 contraction chunks
_AX = 64.0           # scale for xl-cross fp8 pass
_AK = 32.0           # scale for kl-cross fp8 pass
_NPS = 4             # rotating score PSUM buffers (4 x 2 banks = all 8)

_cached = None


def _build():
    import concourse.mybir as mybir
    from concourse import bacc
    from concourse.bass import IndirectOffsetOnAxis
    from concourse.tile import TileContext

    f32 = mybir.dt.float32
    f16 = mybir.dt.float16
    f8 = mybir.dt.float8e4
    u32 = mybir.dt.uint32
    bf16 = mybir.dt.bfloat16
    DR = mybir.MatmulPerfMode.DoubleRow

    nc = bacc.Bacc("TRN2", target_bir_lowering=False, debug=False,
                   num_devices=_NCORES)
    xh16 = nc.dram_tensor("xh16", [_D, _BL], f16, kind="ExternalInput")
    xl8 = nc.dram_tensor("xl8", [_D, _BL], f8, kind="ExternalInput")
    xf8 = nc.dram_tensor("xf8", [_D, _BL], f8, kind="ExternalInput")
    k16 = nc.dram_tensor("k16", [_D, _K], f16, kind="ExternalInput")
    k8 = nc.dram_tensor("k8", [_D, _K], f8, kind="ExternalInput")
    kl8 = nc.dram_tensor("kl8", [_D, _K], f8, kind="ExternalInput")
    biasf = nc.dram_tensor("biasf", [1, _K], f32, kind="ExternalInput")
    vals = nc.dram_tensor("vals", [_K, _D], f16, kind="ExternalInput")
    out = nc.dram_tensor("out", [_BL, _D], f16, kind="ExternalOutput")

    xh3 = xh16.rearrange("(do p) b -> p do b", p=_P)    # [128, 4, 8192]
    xl4 = xl8.rearrange("(c j p) b -> p c j b", p=_P, c=2, j=2)
    xf4 = xf8.rearrange("(c j p) b -> p c j b", p=_P, c=2, j=2)
    k16_3 = k16.rearrange("(do p) k -> p do k", p=_P)   # [128, 4, 1024]
    k8_4 = k8.rearrange("(c j p) k -> p c j k", p=_P, c=2, j=2)
    kl8_4 = kl8.rearrange("(c j p) k -> p c j k", p=_P, c=2, j=2)

    with TileContext(nc) as tc:
        with (
            tc.tile_pool(name="const", bufs=1) as cpool,
            tc.tile_pool(name="xp", bufs=3) as xpool,
            tc.tile_pool(name="warm", bufs=1) as warmpool,
            tc.tile_pool(name="st", bufs=4) as stpool,
            tc.tile_pool(name="gp", bufs=4) as gpool,
            tc.tile_pool(name="ps", bufs=_NPS, space="PSUM") as pspool,
        ):
            # All input loads share the Sync engine's HWDGE queue so they
            # drain in consumption order (tile0's needs first); split queues
            # let the x-block prefetch flood starve the tiny const loads.
            bias_row = cpool.tile([1, _K], f32)
            bias_sb = cpool.tile([_P, _K], f32)
            k16_sb = cpool.tile([_P, _DC, _K], f16)
            k8_sb = cpool.tile([_P, 2, 2, _K], f8)
            kl8_sb = cpool.tile([_P, 2, 2, _K], f8)
            # Loads queue in tile0's exact consumption order: bias, k16 h0
            # + x block0 (first 4 hi matmuls), k16 h1 (next 4), then the
            # fp8 cross operands.
            nc.sync.dma_start(bias_row[:], biasf[:, :])
            nc.sync.dma_start(k16_sb[:, :, 0:512], k16_3[:, :, 0:512])

            xh_t0 = xpool.tile([_P, _DC, _BBLK], f16, tag="xh")
            xl_t0 = xpool.tile([_P, 2, 2, _BBLK], f8, tag="xl")
            xf_t0 = xpool.tile([_P, 2, 2, _BBLK], f8, tag="xf")
            nc.sync.dma_start(xh_t0[:, :, :_BT], xh3[:, :, 0:_BT])
            nc.sync.dma_start(xl_t0[:, :, :, :_BT], xl4[:, :, :, 0:_BT])
            nc.sync.dma_start(xf_t0[:, :, :, :_BT], xf4[:, :, :, 0:_BT])

            nc.sync.dma_start(k16_sb[:, :, 512:1024], k16_3[:, :, 512:1024])
            nc.sync.dma_start(k8_sb[:, :, :, 0:512], k8_4[:, :, :, 0:512])
            nc.sync.dma_start(kl8_sb[:, :, :, 0:512], kl8_4[:, :, :, 0:512])

            # Warmup operands memset on GpSimd (its engine comes up ~1.5us
            # before VectorE).
            ones = warmpool.tile([1, _P], f32)
            nc.gpsimd.memset(ones[:], 1.0)
            wlhs = warmpool.tile([_P, _P], bf16)
            nc.gpsimd.memset(wlhs[:], 0.0)
            wrhs = warmpool.tile([_P, 512], bf16)
            nc.gpsimd.memset(wrhs[:], 0.0)

            # Broadcast the 4KB bias row to all 128 partitions with a pair
            # of K=1 fp32 matmuls (ones.T @ bias_row, exact), then copy
            # PSUM -> SBUF for the later tiles. The two cold fp32 4-pass
            # matmuls double as the PE clock (HAM) warmup; tile0 skips its
            # bias copy and accumulates straight onto this PSUM tile,
            # whose has_written bits the broadcast matmuls set. The other
            # rotating score PSUM buffers are pre-touched with start=True
            # zero matmuls for the same reason: steady-state tiles never
            # use start=True (the bias is pre-written by ScalarE and
            # matmuls accumulate onto it), and a PE write with
            # has_written=0 would overwrite the bias instead of
            # accumulating.
            # The pre-touches have no DMA dependency, so they run first
            # and hide under the bias-row DMA wait (ring bring-up takes
            # ~4us); the broadcast matmuls follow as soon as the 4KB bias
            # lands.
            for b in range(_NPS - 1):
                wtile = pspool.tile([_P, _K], f32, tag="ps")
                nc.tensor.matmul(wtile[:, 0:512], lhsT=wlhs[:],
                                 rhs=wrhs[:], start=True, stop=True)
                nc.tensor.matmul(wtile[:, 512:1024], lhsT=wlhs[:],
                                 rhs=wrhs[:], start=True, stop=True)
            btile = pspool.tile([_P, _K], f32, tag="ps")
            nc.tensor.matmul(btile[:, 0:512], lhsT=ones[:],
                             rhs=bias_row[:, 0:512], start=True, stop=True)
            nc.tensor.matmul(btile[:, 512:1024], lhsT=ones[:],
                             rhs=bias_row[:, 512:1024], start=True, stop=True)
            # Two half copies so tile0's h0 matmuls (write-after-read on
            # btile) only wait for the h0 half.
            nc.scalar.copy(out=bias_sb[:, 0:512], in_=btile[:, 0:512])
            nc.scalar.copy(out=bias_sb[:, 512:1024], in_=btile[:, 512:1024])

            # Remaining const halves, queued behind tile0's critical loads
            # but ahead of the bulk x prefetch below.
            nc.sync.dma_start(k8_sb[:, :, :, 512:1024], k8_4[:, :, :, 512:1024])
            nc.sync.dma_start(kl8_sb[:, :, :, 512:1024], kl8_4[:, :, :, 512:1024])

            # Small leading blocks so early tiles are not gated on 2MB
            # x-block transfers while the DMA rings are still ramping.
            blocks = [(0, 128), (128, 128), (256, 256)]
            off = 512
            while off < _BL:
                w = min(_BBLK, _BL - off)
                blocks.append((off, w))
                off += w

            for bi, (boff, bw) in enumerate(blocks):
                if bi == 0:
                    xh_t, xl_t, xf_t = xh_t0, xl_t0, xf_t0
                else:
                    xh_t = xpool.tile([_P, _DC, _BBLK], f16, tag="xh")
                    xl_t = xpool.tile([_P, 2, 2, _BBLK], f8, tag="xl")
                    xf_t = xpool.tile([_P, 2, 2, _BBLK], f8, tag="xf")
                    nc.sync.dma_start(xh_t[:, :, :bw], xh3[:, :, boff:boff + bw])
                    nc.sync.dma_start(xl_t[:, :, :, :bw], xl4[:, :, :, boff:boff + bw])
                    nc.sync.dma_start(xf_t[:, :, :, :bw], xf4[:, :, :, boff:boff + bw])

                for sub in range(bw // _BT):
                    bt = boff // _BT + sub
                    bsl = slice(sub * _BT, (sub + 1) * _BT)
                    if bt == 0:
                        ps = btile
                    else:
                        ps = pspool.tile([_P, _K], f32, tag="ps")
                    hs = [slice(0, 512), slice(512, 1024)]
                    # ScalarE pre-writes the exact f32 -|k|^2 bias into PSUM;
                    # all matmuls then accumulate onto it (start=False; the
                    # has_written bits were set once by the warmup matmuls).
                    # Tile0 runs on the bias-broadcast PSUM buffer, which
                    # already holds the bias, so it skips the copy.
                    # Matmuls grouped by PE mode (fp16 then fp8-DR) to
                    # minimize mode-switch stalls; within the fp16 group,
                    # dc outer / h inner so consecutive matmuls share the
                    # same stationary operand.
                    if bt > 0:
                        nc.scalar.copy(out=ps[:], in_=bias_sb[:])
                    for dc in range(_DC):
                        for h in range(2):
                            nc.tensor.matmul(ps[:, hs[h]], lhsT=xh_t[:, dc, bsl],
                                             rhs=k16_sb[:, dc, hs[h]],
                                             start=False, stop=False,
                                             skip_group_check=True)
                    for c in range(2):
                        for h in range(2):
                            nc.tensor.matmul(ps[:, hs[h]], lhsT=xl_t[:, c, :, bsl],
                                             rhs=k8_sb[:, c, :, hs[h]],
                                             perf_mode=DR,
                                             start=False, stop=False,
                                             skip_group_check=True)
                    for c in range(2):
                        for h in range(2):
                            nc.tensor.matmul(ps[:, hs[h]], lhsT=xf_t[:, c, :, bsl],
                                             rhs=kl8_sb[:, c, :, hs[h]],
                                             perf_mode=DR,
                                             start=False, stop=(c == 1),
                                             skip_group_check=True)
                    mx = stpool.tile([_P, 8], f32)
                    nc.vector.max(out=mx[:], in_=ps[:])
                    idx = stpool.tile([_P, 8], u32)
                    nc.vector.max_index(out=idx[:], in_max=mx[:], in_values=ps[:])

                    g = gpool.tile([_P, _D], f16)
                    nc.gpsimd.indirect_dma_start(
                        out=g[:],
                        out_offset=None,
                        in_=vals[:, :],
                        in_offset=IndirectOffsetOnAxis(ap=idx[:, :1], axis=0),
                    )
                    # Rotate output stores across all three DMA-capable
                    # queues: a single queue drains the last few tiles'
                    # 128KB outs serially (~1.1us each) after the final
                    # gather, adding ~2us to the tail.
                    outq = (nc.scalar, nc.sync, nc.gpsimd)[bt % 3]
                    outq.dma_start(out[bt * _BT:(bt + 1) * _BT, :], g[:])

    nc.compile()
    return nc


def _get_nc():
    global _cached
    if _cached is None:
        _cached = _build()
    return _cached


def _fp8(a):
    import ml_dtypes

    return np.clip(a, -240.0, 240.0).astype(ml_dtypes.float8_e4m3)


def _prepare_in_maps(x, keys, values):
    x = np.asarray(x, dtype=np.float32)
    keys = np.asarray(keys, dtype=np.float32)
    values = np.asarray(values, dtype=np.float32)

    kT = np.ascontiguousarray((2.0 * keys).T)            # [512, 1024] f32
    k16 = kT.astype(np.float16)
    kl = kT - k16.astype(np.float32)
    k8 = _fp8(kT / _AX)
    kl8 = _fp8(kl * _AK)

    k2 = np.einsum("kd,kd->k", keys.astype(np.float64),
                   keys.astype(np.float64))
    biasf = np.ascontiguousarray((-k2).astype(np.float32)[None, :])
    vals16 = values.astype(np.float16)

    in_maps = []
    for c in range(_NCORES):
        xs = np.ascontiguousarray(x[c * _BL:(c + 1) * _BL].T)  # [512, 8192]
        xh16 = xs.astype(np.float16)
        xl = xs - xh16.astype(np.float32)
        xl8 = _fp8(xl * _AX)
        xf8 = _fp8(xs / _AK)
        in_maps.append({
            "xh16": xh16, "xl8": xl8, "xf8": xf8,
            "k16": k16, "k8": k8, "kl8": kl8,
            "biasf": biasf, "vals": vals16,
        })
    return in_maps


def kernel(x, keys, values):
    from concourse.bass_utils import run_bass_kernel_spmd

    nc = _get_nc()
    in_maps = _prepare_in_maps(x, keys, values)
    res = run_bass_kernel_spmd(nc, in_maps, core_ids=list(range(_NCORES)))
    return np.concatenate([r["out"] for r in res.results],
                          axis=0).astype(np.float32)


# revision 28
# speedup vs baseline: 1.0130x; 1.0130x over previous
"""VQ codebook lookup kernel for Trainium2 (8 NeuronCores, data-parallel).

Computes out[b] = values[argmin_k ||x[b] - keys[k]||] for
x [65536, 512], keys/values [1024, 512] fp32.

Strategy (per core, batch shard of 8192 rows):
  - argmin distance == argmax s = 2*x.k - |k|^2 (sqrt and the |x|^2 row
    offset do not change the argmin).
  - Per 128-row tile, s[128, 1024] accumulates in PSUM from:
      1. ScalarE pre-writes the exact f32 -|k|^2 bias into the PSUM tile;
         all matmuls use start=False and accumulate onto it (warmup
         matmuls pre-set the has_written bits of the 4 rotating PSUM
         buffers once, so the PE accumulates instead of overwriting).
      2. 8x fp16 matmuls (K=128, N=512): fp16(x).T @ fp16(2k) hi*hi term.
         The PE multiplies fp16 operands exactly (fp22 internally) and
         accumulates in fp32.
      3. 8x fp8-e4m3 DoubleRow matmuls (K=256 packed, N=512) for the two
         cross terms: fp8(xl*64) . fp8(2k/64) and fp8(x/32) . fp8(kl*32),
         where xl = x - fp16(x), kl = 2k - fp16(2k); scales keep the
         operands inside e4m3 range. DoubleRow packs 2 contraction rows
         per PE cell, halving the matmul count for these passes.
    A matmul instruction costs ~N cycles regardless of dtype, so the tile
    costs 16 matmuls vs 24 for the bf16 hi/lo x3 scheme (1.5x less PE
    time); matmuls are grouped by PE mode (fp16, then DoubleRow) to
    minimize mode-switch stalls. Numpy-sim of this exact arithmetic
    flips 1 of 65536 argmaxes (rel err 5.5e-3, gate 2e-2).
  - DVE per tile: MAX8 + FIND_INDEX8 read straight from PSUM (no
    PSUM->SBUF move), then a gpsimd indirect-DMA gathers fp16 values
    rows and the result is written out from the Scalar queue; the host
    upcasts the fp16 output to f32 (values fp16 rounding adds ~1e-3 rel,
    far under the gate, and halves the gather+store HBM traffic).

Schedule (worth ~8us vs the naive version): all input loads share one
DMA queue in tile0's consumption order so the bulk x prefetch cannot
starve tile0's operands; the PSUM pre-touches run first (no DMA deps,
hidden under the ~4us DMA-ring bring-up); the bias ships as a 4KB row
that two cold K=1 fp32 matmuls broadcast across partitions (doubling
as HAM clock warmup) with tile0 accumulating directly onto that PSUM
tile; 4 rotating PSUM score buffers (all 8 banks) decouple the PE from
the DVE argmax + gather pipeline; leading x blocks are small
(128/128/256 rows) so early tiles aren't gated on 2MB transfers while
the DMA rings ramp; output stores alternate between the two HWDGE
queues so the final tiles' stores drain in parallel.

Measured on TRN2: HW exec ~251.5us (baseline 259.1us), rel err
5.47e-3, PE matmul stream ~99% dense at the 1-column/cycle roofline.
Note the chip sometimes drops to a lower power state under sustained
load (every engine ~18% slower); timings are from unthrottled runs
(steady-state N=512 matmul ~216ns; ~259ns when throttled).
"""

import numpy as np

_B = 65536
_D = 512
_K = 1024
_NCORES = 8
_BL = _B // _NCORES  # 8192 rows per core
_P = 128
_BBLK = 512          # b columns loaded per DMA
_BT = 128            # b rows per matmul tile (PSUM partition dim)
_DC = _D // _P       # 4 contraction chunks
_AX = 64.0           # scale for xl-cross fp8 pass
_AK = 32.0           # scale for kl-cross fp8 pass
_NPS = 4             # rotating score PSUM buffers (4 x 2 banks = all 8)

_cached = None


def _build():
    import concourse.mybir as mybir
    from concourse import bacc
    from concourse.bass import IndirectOffsetOnAxis
    from concourse.tile import TileContext

    f32 = mybir.dt.float32
    f16 = mybir.dt.float16
    f8 = mybir.dt.float8e4
    u32 = mybir.dt.uint32
    bf16 = mybir.dt.bfloat16
    DR = mybir.MatmulPerfMode.DoubleRow

    nc = bacc.Bacc("TRN2", target_bir_lowering=False, debug=False,
                   num_devices=_NCORES)
    xh16 = nc.dram_tensor("xh16", [_D, _BL], f16, kind="ExternalInput")
    xl8 = nc.dram_tensor("xl8", [_D, _BL], f8, kind="ExternalInput")
    xf8 = nc.dram_tensor("xf8", [_D, _BL], f8, kind="ExternalInput")
    k16 = nc.dram_tensor("k16", [_D, _K], f16, kind="ExternalInput")
    k8 = nc.dram_tensor("k8", [_D, _K], f8, kind="ExternalInput")
    kl8 = nc.dram_tensor("kl8", [_D, _K], f8, kind="ExternalInput")
    biasf = nc.dram_tensor("biasf", [1, _K], f32, kind="ExternalInput")
    vals = nc.dram_tensor("vals", [_K, _D], f16, kind="ExternalInput")
    out = nc.dram_tensor("out", [_BL, _D], f16, kind="ExternalOutput")

    xh3 = xh16.rearrange("(do p) b -> p do b", p=_P)    # [128, 4, 8192]
    xl4 = xl8.rearrange("(c j p) b -> p c j b", p=_P, c=2, j=2)
    xf4 = xf8.rearrange("(c j p) b -> p c j b", p=_P, c=2, j=2)
    k16_3 = k16.rearrange("(do p) k -> p do k", p=_P)   # [128, 4, 1024]
    k8_4 = k8.rearrange("(c j p) k -> p c j k", p=_P, c=2, j=2)
    kl8_4 = kl8.rearrange("(c j p) k -> p c j k", p=_P, c=2, j=2)

    with TileContext(nc) as tc:
        with (
            tc.tile_pool(name="const", bufs=1) as cpool,
            tc.tile_pool(name="xp", bufs=3) as xpool,
            tc.tile_pool(name="warm", bufs=1) as warmpool,
            tc.tile_pool(name="st", bufs=4) as stpool,
            tc.tile_pool(name="gp", bufs=4) as gpool,
            tc.tile_pool(name="ps", bufs=_NPS, space="PSUM") as pspool,
        ):
            # All input loads share the Sync engine's HWDGE queue so they
            # drain in consumption order (tile0's needs first); split queues
            # let the x-block prefetch flood starve the tiny const loads.
            bias_row = cpool.tile([1, _K], f32)
            bias_sb = cpool.tile([_P, _K], f32)
            k16_sb = cpool.tile([_P, _DC, _K], f16)
            k8_sb = cpool.tile([_P, 2, 2, _K], f8)
            kl8_sb = cpool.tile([_P, 2, 2, _K], f8)
            # Loads queue in tile0's exact consumption order: bias, k16 h0
            # + x block0 (first 4 hi matmuls), k16 h1 (next 4), then the
            # fp8 cross operands.
            nc.sync.dma_start(bias_row[:], biasf[:, :])
            nc.sync.dma_start(k16_sb[:, :, 0:512], k16_3[:, :, 0:512])

            xh_t0 = xpool.tile([_P, _DC, _BBLK], f16, tag="xh")
            xl_t0 = xpool.tile([_P, 2, 2, _BBLK], f8, tag="xl")
            xf_t0 = xpool.tile([_P, 2, 2, _BBLK], f8, tag="xf")
            nc.sync.dma_start(xh_t0[:, :, :_BT], xh3[:, :, 0:_BT])
            nc.sync.dma_start(xl_t0[:, :, :, :_BT], xl4[:, :, :, 0:_BT])
            nc.sync.dma_start(xf_t0[:, :, :, :_BT], xf4[:, :, :, 0:_BT])

            nc.sync.dma_start(k16_sb[:, :, 512:1024], k16_3[:, :, 512:1024])
            nc.sync.dma_start(k8_sb[:, :, :, 0:512], k8_4[:, :, :, 0:512])
            nc.sync.dma_start(kl8_sb[:, :, :, 0:512], kl8_4[:, :, :, 0:512])

            # Warmup operands memset on GpSimd (its engine comes up ~1.5us
            # before VectorE).
            ones = warmpool.tile([1, _P], f32)
            nc.gpsimd.memset(ones[:], 1.0)
            wlhs = warmpool.tile([_P, _P], bf16)
            nc.gpsimd.memset(wlhs[:], 0.0)
            wrhs = warmpool.tile([_P, 512], bf16)
            nc.gpsimd.memset(wrhs[:], 0.0)

            # Broadcast the 4KB bias row to all 128 partitions with a pair
            # of K=1 fp32 matmuls (ones.T @ bias_row, exact), then copy
            # PSUM -> SBUF for the later tiles. The two cold fp32 4-pass
            # matmuls double as the PE clock (HAM) warmup; tile0 skips its
            # bias copy and accumulates straight onto this PSUM tile,
            # whose has_written bits the broadcast matmuls set. The other
            # rotating score PSUM buffers are pre-touched with start=True
            # zero matmuls for the same reason: steady-state tiles never
            # use start=True (the bias is pre-written by ScalarE and
            # matmuls accumulate onto it), and a PE write with
            # has_written=0 would overwrite the bias instead of
            # accumulating.
            # The pre-touches have no DMA dependency, so they run first
            # and hide under the bias-row DMA wait (ring bring-up takes
            # ~4us); the broadcast matmuls follow as soon as the 4KB bias
            # lands.
            for b in range(_NPS - 1):
                wtile = pspool.tile([_P, _K], f32, tag="ps")
                nc.tensor.matmul(wtile[:, 0:512], lhsT=wlhs[:],
                                 rhs=wrhs[:], start=True, stop=True)
                nc.tensor.matmul(wtile[:, 512:1024], lhsT=wlhs[:],
                                 rhs=wrhs[:], start=True, stop=True)
            btile = pspool.tile([_P, _K], f32, tag="ps")
            nc.tensor.matmul(btile[:, 0:512], lhsT=ones[:],
                             rhs=bias_row[:, 0:512], start=True, stop=True)
            nc.tensor.matmul(btile[:, 512:1024], lhsT=ones[:],
                             rhs=bias_row[:, 512:1024], start=True, stop=True)
            # Two half copies so tile0's h0 matmuls (write-after-read on
            # btile) only wait for the h0 half.
            nc.scalar.copy(out=bias_sb[:, 0:512], in_=btile[:, 0:512])
            nc.scalar.copy(out=bias_sb[:, 512:1024], in_=btile[:, 512:1024])

            # Remaining const halves, queued behind tile0's critical loads
            # but ahead of the bulk x prefetch below.
            nc.sync.dma_start(k8_sb[:, :, :, 512:1024], k8_4[:, :, :, 512:1024])
            nc.sync.dma_start(kl8_sb[:, :, :, 512:1024], kl8_4[:, :, :, 512:1024])

            # Small leading blocks so early tiles are not gated on 2MB
            # x-block transfers while the DMA rings are still ramping.
            blocks = [(0, 128), (128, 128), (256, 256)]
            off = 512
            while off < _BL:
                w = min(_BBLK, _BL - off)
                blocks.append((off, w))
                off += w

            for bi, (boff, bw) in enumerate(blocks):
                if bi == 0:
                    xh_t, xl_t, xf_t = xh_t0, xl_t0, xf_t0
                else:
                    xh_t = xpool.tile([_P, _DC, _BBLK], f16, tag="xh")
                    xl_t = xpool.tile([_P, 2, 2, _BBLK], f8, tag="xl")
                    xf_t = xpool.tile([_P, 2, 2, _BBLK], f8, tag="xf")
                    nc.sync.dma_start(xh_t[:, :, :bw], xh3[:, :, boff:boff + bw])
                    nc.sync.dma_start(xl_t[:, :, :, :bw], xl4[:, :, :, boff:boff + bw])
                    nc.sync.dma_start(xf_t[:, :, :, :bw], xf4[:, :, :, boff:boff + bw])

                for sub in range(bw // _BT):
                    bt = boff // _BT + sub
                    bsl = slice(sub * _BT, (sub + 1) * _BT)
                    if bt == 0:
                        ps = btile
                    else:
                        ps = pspool.tile([_P, _K], f32, tag="ps")
                    hs = [slice(0, 512), slice(512, 1024)]
                    # ScalarE pre-writes the exact f32 -|k|^2 bias into PSUM;
                    # all matmuls then accumulate onto it (start=False; the
                    # has_written bits were set once by the warmup matmuls).
                    # Tile0 runs on the bias-broadcast PSUM buffer, which
                    # already holds the bias, so it skips the copy.
                    # Matmuls grouped by PE mode (fp16 then fp8-DR) to
                    # minimize mode-switch stalls; within the fp16 group,
                    # dc outer / h inner so consecutive matmuls share the
                    # same stationary operand.
                    if bt > 0:
                        nc.scalar.copy(out=ps[:], in_=bias_sb[:])
                    for dc in range(_DC):
                        for h in range(2):
                            nc.tensor.matmul(ps[:, hs[h]], lhsT=xh_t[:, dc, bsl],
                                             rhs=k16_sb[:, dc, hs[h]],
                                             start=False, stop=False,
                                             skip_group_check=True)
                    for c in range(2):
                        for h in range(2):
                            nc.tensor.matmul(ps[:, hs[h]], lhsT=xl_t[:, c, :, bsl],
                                             rhs=k8_sb[:, c, :, hs[h]],
                                             perf_mode=DR,
                                             start=False, stop=False,
                                             skip_group_check=True)
                    for c in range(2):
                        for h in range(2):
                            nc.tensor.matmul(ps[:, hs[h]], lhsT=xf_t[:, c, :, bsl],
                                             rhs=kl8_sb[:, c, :, hs[h]],
                                             perf_mode=DR,
                                             start=False, stop=(c == 1),
                                             skip_group_check=True)
                    mx = stpool.tile([_P, 8], f32)
                    nc.vector.max(out=mx[:], in_=ps[:])
                    idx = stpool.tile([_P, 8], u32)
                    nc.vector.max_index(out=idx[:], in_max=mx[:], in_values=ps[:])

                    g = gpool.tile([_P, _D], f16)
                    nc.gpsimd.indirect_dma_start(
                        out=g[:],
                        out_offset=None,
                        in_=vals[:, :],
                        in_offset=IndirectOffsetOnAxis(ap=idx[:, :1], axis=0),
                    )
                    # Alternate output stores between the two HWDGE queues:
                    # a single queue drains the last few tiles' 128KB outs
                    # serially (~1.1us each) after the final gather, adding
                    # ~2us to the tail. (Only SP/Activation issue cheaply;
                    # gpsimd dma_start costs ~650ns of engine time and
                    # delays the gathers - measured 4us slower.)
                    outq = (nc.scalar, nc.sync)[bt % 2]
                    outq.dma_start(out[bt * _BT:(bt + 1) * _BT, :], g[:])

    nc.compile()
    return nc


def _get_nc():
    global _cached
    if _cached is None:
        _cached = _build()
    return _cached


def _fp8(a):
    import ml_dtypes

    return np.clip(a, -240.0, 240.0).astype(ml_dtypes.float8_e4m3)


def _prepare_in_maps(x, keys, values):
    x = np.asarray(x, dtype=np.float32)
    keys = np.asarray(keys, dtype=np.float32)
    values = np.asarray(values, dtype=np.float32)

    kT = np.ascontiguousarray((2.0 * keys).T)            # [512, 1024] f32
    k16 = kT.astype(np.float16)
    kl = kT - k16.astype(np.float32)
    k8 = _fp8(kT / _AX)
    kl8 = _fp8(kl * _AK)

    k2 = np.einsum("kd,kd->k", keys.astype(np.float64),
                   keys.astype(np.float64))
    biasf = np.ascontiguousarray((-k2).astype(np.float32)[None, :])
    vals16 = values.astype(np.float16)

    in_maps = []
    for c in range(_NCORES):
        xs = np.ascontiguousarray(x[c * _BL:(c + 1) * _BL].T)  # [512, 8192]
        xh16 = xs.astype(np.float16)
        xl = xs - xh16.astype(np.float32)
        xl8 = _fp8(xl * _AX)
        xf8 = _fp8(xs / _AK)
        in_maps.append({
            "xh16": xh16, "xl8": xl8, "xf8": xf8,
            "k16": k16, "k8": k8, "kl8": kl8,
            "biasf": biasf, "vals": vals16,
        })
    return in_maps


def kernel(x, keys, values):
    from concourse.bass_utils import run_bass_kernel_spmd

    nc = _get_nc()
    in_maps = _prepare_in_maps(x, keys, values)
    res = run_bass_kernel_spmd(nc, in_maps, core_ids=list(range(_NCORES)))
    return np.concatenate([r["out"] for r in res.results],
                          axis=0).astype(np.float32)


# revision 29
# speedup vs baseline: 1.0172x; 1.0041x over previous
"""VQ codebook lookup kernel for Trainium2 (8 NeuronCores, data-parallel).

Computes out[b] = values[argmin_k ||x[b] - keys[k]||] for
x [65536, 512], keys/values [1024, 512] fp32.

Strategy (per core, batch shard of 8192 rows):
  - argmin distance == argmax s = 2*x.k - |k|^2 (sqrt and the |x|^2 row
    offset do not change the argmin).
  - Per 128-row tile, s[128, 1024] accumulates in PSUM from:
      1. ScalarE pre-writes the exact f32 -|k|^2 bias into the PSUM tile;
         all matmuls use start=False and accumulate onto it (warmup
         matmuls pre-set the has_written bits of the 4 rotating PSUM
         buffers once, so the PE accumulates instead of overwriting).
      2. 8x fp16 matmuls (K=128, N=512): fp16(x).T @ fp16(2k) hi*hi term.
         The PE multiplies fp16 operands exactly (fp22 internally) and
         accumulates in fp32.
      3. 8x fp8-e4m3 DoubleRow matmuls (K=256 packed, N=512) for the two
         cross terms: fp8(xl*64) . fp8(2k/64) and fp8(x/32) . fp8(kl*32),
         where xl = x - fp16(x), kl = 2k - fp16(2k); scales keep the
         operands inside e4m3 range. DoubleRow packs 2 contraction rows
         per PE cell, halving the matmul count for these passes.
    A matmul instruction costs ~N cycles regardless of dtype, so the tile
    costs 16 matmuls vs 24 for the bf16 hi/lo x3 scheme (1.5x less PE
    time); matmuls are grouped by PE mode (fp16, then DoubleRow) to
    minimize mode-switch stalls. Numpy-sim of this exact arithmetic
    flips 1 of 65536 argmaxes (rel err 5.5e-3, gate 2e-2).
  - DVE per tile: MAX8 + FIND_INDEX8 read straight from PSUM (no
    PSUM->SBUF move), then a gpsimd indirect-DMA gathers fp16 values
    rows and the result is written out from the Scalar queue; the host
    upcasts the fp16 output to f32 (values fp16 rounding adds ~1e-3 rel,
    far under the gate, and halves the gather+store HBM traffic).

Schedule (worth ~8us vs the naive version): all input loads share one
DMA queue in tile0's consumption order so the bulk x prefetch cannot
starve tile0's operands; the PSUM pre-touches run first (no DMA deps,
hidden under the ~4us DMA-ring bring-up); the bias ships as a 4KB row
that two cold K=1 fp32 matmuls broadcast across partitions (doubling
as HAM clock warmup) with tile0 accumulating directly onto that PSUM
tile; 4 rotating PSUM score buffers (all 8 banks) decouple the PE from
the DVE argmax + gather pipeline; leading x blocks are small
(128/128/256 rows) so early tiles aren't gated on 2MB transfers while
the DMA rings ramp; output stores alternate between the two HWDGE
queues so the final tiles' stores drain in parallel.

Measured on TRN2: HW exec ~251.5us (baseline 259.1us), rel err
5.47e-3, PE matmul stream ~99% dense at the 1-column/cycle roofline.
Note the chip sometimes drops to a lower power state under sustained
load (every engine ~18% slower); timings are from unthrottled runs
(steady-state N=512 matmul ~216ns; ~259ns when throttled).
"""

import numpy as np

_B = 65536
_D = 512
_K = 1024
_NCORES = 8
_BL = _B // _NCORES  # 8192 rows per core
_P = 128
_BBLK = 512          # b columns loaded per DMA
_BT = 128            # b rows per matmul tile (PSUM partition dim)
_DC = _D // _P       # 4 contraction chunks
_AX = 64.0           # scale for xl-cross fp8 pass
_AK = 32.0           # scale for kl-cross fp8 pass
_NPS = 4             # rotating score PSUM buffers (4 x 2 banks = all 8)

_cached = None


def _build():
    import concourse.mybir as mybir
    from concourse import bacc
    from concourse.bass import IndirectOffsetOnAxis
    from concourse.tile import TileContext

    f32 = mybir.dt.float32
    f16 = mybir.dt.float16
    f8 = mybir.dt.float8e4
    u32 = mybir.dt.uint32
    bf16 = mybir.dt.bfloat16
    DR = mybir.MatmulPerfMode.DoubleRow

    nc = bacc.Bacc("TRN2", target_bir_lowering=False, debug=False,
                   num_devices=_NCORES)
    xh16 = nc.dram_tensor("xh16", [_D, _BL], f16, kind="ExternalInput")
    xl8 = nc.dram_tensor("xl8", [_D, _BL], f8, kind="ExternalInput")
    xf8 = nc.dram_tensor("xf8", [_D, _BL], f8, kind="ExternalInput")
    k16 = nc.dram_tensor("k16", [_D, _K], f16, kind="ExternalInput")
    k8 = nc.dram_tensor("k8", [_D, _K], f8, kind="ExternalInput")
    kl8 = nc.dram_tensor("kl8", [_D, _K], f8, kind="ExternalInput")
    biasf = nc.dram_tensor("biasf", [1, _K], f32, kind="ExternalInput")
    vals = nc.dram_tensor("vals", [_K, _D], f16, kind="ExternalInput")
    out = nc.dram_tensor("out", [_BL, _D], f16, kind="ExternalOutput")

    xh3 = xh16.rearrange("(do p) b -> p do b", p=_P)    # [128, 4, 8192]
    xl4 = xl8.rearrange("(c j p) b -> p c j b", p=_P, c=2, j=2)
    xf4 = xf8.rearrange("(c j p) b -> p c j b", p=_P, c=2, j=2)
    k16_3 = k16.rearrange("(do p) k -> p do k", p=_P)   # [128, 4, 1024]
    k8_4 = k8.rearrange("(c j p) k -> p c j k", p=_P, c=2, j=2)
    kl8_4 = kl8.rearrange("(c j p) k -> p c j k", p=_P, c=2, j=2)

    with TileContext(nc) as tc:
        with (
            tc.tile_pool(name="const", bufs=1) as cpool,
            tc.tile_pool(name="xp", bufs=4) as xpool,
            tc.tile_pool(name="warm", bufs=1) as warmpool,
            tc.tile_pool(name="st", bufs=4) as stpool,
            tc.tile_pool(name="gp", bufs=4) as gpool,
            tc.tile_pool(name="ps", bufs=_NPS, space="PSUM") as pspool,
        ):
            # All input loads share the Sync engine's HWDGE queue so they
            # drain in consumption order (tile0's needs first); split queues
            # let the x-block prefetch flood starve the tiny const loads.
            bias_row = cpool.tile([1, _K], f32)
            bias_sb = cpool.tile([_P, _K], f32)
            k16_sb = cpool.tile([_P, _DC, _K], f16)
            k8_sb = cpool.tile([_P, 2, 2, _K], f8)
            kl8_sb = cpool.tile([_P, 2, 2, _K], f8)
            # Loads queue in tile0's exact consumption order: bias, k16 h0
            # + x block0 (first 4 hi matmuls), k16 h1 (next 4), then the
            # fp8 cross operands.
            nc.sync.dma_start(bias_row[:], biasf[:, :])
            nc.sync.dma_start(k16_sb[:, :, 0:512], k16_3[:, :, 0:512])

            xh_t0 = xpool.tile([_P, _DC, _BBLK], f16, tag="xh")
            xl_t0 = xpool.tile([_P, 2, 2, _BBLK], f8, tag="xl")
            xf_t0 = xpool.tile([_P, 2, 2, _BBLK], f8, tag="xf")
            nc.sync.dma_start(xh_t0[:, :, :_BT], xh3[:, :, 0:_BT])
            nc.sync.dma_start(xl_t0[:, :, :, :_BT], xl4[:, :, :, 0:_BT])
            nc.sync.dma_start(xf_t0[:, :, :, :_BT], xf4[:, :, :, 0:_BT])

            nc.sync.dma_start(k16_sb[:, :, 512:1024], k16_3[:, :, 512:1024])
            nc.sync.dma_start(k8_sb[:, :, :, 0:512], k8_4[:, :, :, 0:512])
            nc.sync.dma_start(kl8_sb[:, :, :, 0:512], kl8_4[:, :, :, 0:512])

            # Warmup operands memset on GpSimd (its engine comes up ~1.5us
            # before VectorE).
            ones = warmpool.tile([1, _P], f32)
            nc.gpsimd.memset(ones[:], 1.0)
            wlhs = warmpool.tile([_P, _P], bf16)
            nc.gpsimd.memset(wlhs[:], 0.0)
            wrhs = warmpool.tile([_P, 512], bf16)
            nc.gpsimd.memset(wrhs[:], 0.0)

            # Broadcast the 4KB bias row to all 128 partitions with a pair
            # of K=1 fp32 matmuls (ones.T @ bias_row, exact), then copy
            # PSUM -> SBUF for the later tiles. The two cold fp32 4-pass
            # matmuls double as the PE clock (HAM) warmup; tile0 skips its
            # bias copy and accumulates straight onto this PSUM tile,
            # whose has_written bits the broadcast matmuls set. The other
            # rotating score PSUM buffers are pre-touched with start=True
            # zero matmuls for the same reason: steady-state tiles never
            # use start=True (the bias is pre-written by ScalarE and
            # matmuls accumulate onto it), and a PE write with
            # has_written=0 would overwrite the bias instead of
            # accumulating.
            # The pre-touches have no DMA dependency, so they run first
            # and hide under the bias-row DMA wait (ring bring-up takes
            # ~4us); the broadcast matmuls follow as soon as the 4KB bias
            # lands.
            for b in range(_NPS - 1):
                wtile = pspool.tile([_P, _K], f32, tag="ps")
                nc.tensor.matmul(wtile[:, 0:512], lhsT=wlhs[:],
                                 rhs=wrhs[:], start=True, stop=True)
                nc.tensor.matmul(wtile[:, 512:1024], lhsT=wlhs[:],
                                 rhs=wrhs[:], start=True, stop=True)
            btile = pspool.tile([_P, _K], f32, tag="ps")
            nc.tensor.matmul(btile[:, 0:512], lhsT=ones[:],
                             rhs=bias_row[:, 0:512], start=True, stop=True)
            nc.tensor.matmul(btile[:, 512:1024], lhsT=ones[:],
                             rhs=bias_row[:, 512:1024], start=True, stop=True)
            # Two half copies so tile0's h0 matmuls (write-after-read on
            # btile) only wait for the h0 half.
            nc.scalar.copy(out=bias_sb[:, 0:512], in_=btile[:, 0:512])
            nc.scalar.copy(out=bias_sb[:, 512:1024], in_=btile[:, 512:1024])

            # Remaining const halves, queued behind tile0's critical loads
            # but ahead of the bulk x prefetch below.
            nc.sync.dma_start(k8_sb[:, :, :, 512:1024], k8_4[:, :, :, 512:1024])
            nc.sync.dma_start(kl8_sb[:, :, :, 512:1024], kl8_4[:, :, :, 512:1024])

            # Small leading blocks so early tiles are not gated on 2MB
            # x-block transfers while the DMA rings are still ramping.
            blocks = [(0, 128), (128, 128), (256, 256)]
            off = 512
            while off < _BL:
                w = min(_BBLK, _BL - off)
                blocks.append((off, w))
                off += w

            for bi, (boff, bw) in enumerate(blocks):
                if bi == 0:
                    xh_t, xl_t, xf_t = xh_t0, xl_t0, xf_t0
                else:
                    xh_t = xpool.tile([_P, _DC, _BBLK], f16, tag="xh")
                    xl_t = xpool.tile([_P, 2, 2, _BBLK], f8, tag="xl")
                    xf_t = xpool.tile([_P, 2, 2, _BBLK], f8, tag="xf")
                    nc.sync.dma_start(xh_t[:, :, :bw], xh3[:, :, boff:boff + bw])
                    nc.sync.dma_start(xl_t[:, :, :, :bw], xl4[:, :, :, boff:boff + bw])
                    nc.sync.dma_start(xf_t[:, :, :, :bw], xf4[:, :, :, boff:boff + bw])

                for sub in range(bw // _BT):
                    bt = boff // _BT + sub
                    bsl = slice(sub * _BT, (sub + 1) * _BT)
                    if bt == 0:
                        ps = btile
                    else:
                        ps = pspool.tile([_P, _K], f32, tag="ps")
                    hs = [slice(0, 512), slice(512, 1024)]
                    # ScalarE pre-writes the exact f32 -|k|^2 bias into PSUM;
                    # all matmuls then accumulate onto it (start=False; the
                    # has_written bits were set once by the warmup matmuls).
                    # Tile0 runs on the bias-broadcast PSUM buffer, which
                    # already holds the bias, so it skips the copy.
                    # Matmuls grouped by PE mode (fp16 then fp8-DR) to
                    # minimize mode-switch stalls; within the fp16 group,
                    # dc outer / h inner so consecutive matmuls share the
                    # same stationary operand.
                    if bt > 0:
                        nc.scalar.copy(out=ps[:], in_=bias_sb[:])
                    for dc in range(_DC):
                        for h in range(2):
                            nc.tensor.matmul(ps[:, hs[h]], lhsT=xh_t[:, dc, bsl],
                                             rhs=k16_sb[:, dc, hs[h]],
                                             start=False, stop=False,
                                             skip_group_check=True)
                    for c in range(2):
                        for h in range(2):
                            nc.tensor.matmul(ps[:, hs[h]], lhsT=xl_t[:, c, :, bsl],
                                             rhs=k8_sb[:, c, :, hs[h]],
                                             perf_mode=DR,
                                             start=False, stop=False,
                                             skip_group_check=True)
                    for c in range(2):
                        for h in range(2):
                            nc.tensor.matmul(ps[:, hs[h]], lhsT=xf_t[:, c, :, bsl],
                                             rhs=kl8_sb[:, c, :, hs[h]],
                                             perf_mode=DR,
                                             start=False, stop=(c == 1),
                                             skip_group_check=True)
                    mx = stpool.tile([_P, 8], f32)
                    nc.vector.max(out=mx[:], in_=ps[:])
                    idx = stpool.tile([_P, 8], u32)
                    nc.vector.max_index(out=idx[:], in_max=mx[:], in_values=ps[:])

                    g = gpool.tile([_P, _D], f16)
                    nc.gpsimd.indirect_dma_start(
                        out=g[:],
                        out_offset=None,
                        in_=vals[:, :],
                        in_offset=IndirectOffsetOnAxis(ap=idx[:, :1], axis=0),
                    )
                    # Alternate output stores between the two HWDGE queues:
                    # a single queue drains the last few tiles' 128KB outs
                    # serially (~1.1us each) after the final gather, adding
                    # ~2us to the tail. (Only SP/Activation issue cheaply;
                    # gpsimd dma_start costs ~650ns of engine time and
                    # delays the gathers - measured 4us slower.)
                    outq = (nc.scalar, nc.sync)[bt % 2]
                    outq.dma_start(out[bt * _BT:(bt + 1) * _BT, :], g[:])

    nc.compile()
    return nc


def _get_nc():
    global _cached
    if _cached is None:
        _cached = _build()
    return _cached


def _fp8(a):
    import ml_dtypes

    return np.clip(a, -240.0, 240.0).astype(ml_dtypes.float8_e4m3)


def _prepare_in_maps(x, keys, values):
    x = np.asarray(x, dtype=np.float32)
    keys = np.asarray(keys, dtype=np.float32)
    values = np.asarray(values, dtype=np.float32)

    kT = np.ascontiguousarray((2.0 * keys).T)            # [512, 1024] f32
    k16 = kT.astype(np.float16)
    kl = kT - k16.astype(np.float32)
    k8 = _fp8(kT / _AX)
    kl8 = _fp8(kl * _AK)

    k2 = np.einsum("kd,kd->k", keys.astype(np.float64),
                   keys.astype(np.float64))
    biasf = np.ascontiguousarray((-k2).astype(np.float32)[None, :])
    vals16 = values.astype(np.float16)

    in_maps = []
    for c in range(_NCORES):
        xs = np.ascontiguousarray(x[c * _BL:(c + 1) * _BL].T)  # [512, 8192]
        xh16 = xs.astype(np.float16)
        xl = xs - xh16.astype(np.float32)
        xl8 = _fp8(xl * _AX)
        xf8 = _fp8(xs / _AK)
        in_maps.append({
            "xh16": xh16, "xl8": xl8, "xf8": xf8,
            "k16": k16, "k8": k8, "kl8": kl8,
            "biasf": biasf, "vals": vals16,
        })
    return in_maps


def kernel(x, keys, values):
    from concourse.bass_utils import run_bass_kernel_spmd

    nc = _get_nc()
    in_maps = _prepare_in_maps(x, keys, values)
    res = run_bass_kernel_spmd(nc, in_maps, core_ids=list(range(_NCORES)))
    return np.concatenate([r["out"] for r in res.results],
                          axis=0).astype(np.float32)
